# revision 43
# baseline (speedup 1.0000x reference)
"""Trainium2 Bass kernel for causal linear-attention approximation (bf16 v2).

Reference computation (per batch b, head h):
  q,k = hidden @ Wq|Wk -> (L, F=16);  v = hidden @ Wv -> (L, DH=64)
  ck = k - cummean(k);  cv = v - cummean(v)        (cumsums over seq)
  qK[i,j] = q_i . ck_j   (causal: j<=i)
  s[i] = sum_j qK[i,j]^2 / (2*DH);  qKsq = cumsum_i(s);  den = (i+1)+qKsq
  y = cummean(v) + (qK @ cv) / (sqrt(DH) * den)
  out = concat_heads(y) @ Wo

Distribution: 8 cores = 2 batches x 4 head-groups (3 heads each). Each core
computes a partial (L, D) output = y_heads @ Wo_rows; host sums 4 partials
per batch.

v2 vs baseline:
  - bf16 matmul operands / elementwise tiles (PSUM accumulation stays f32;
    scans carry f32 state).  1 cycle/row matmuls at any N, 2x DVE rate.
  - Block-granular (128-key) C/G running history: only the 16 diagonal
    128x128 blocks of qK are materialized; everything earlier flows through
    C = ck^T ck (16x16) and G = ck^T cv (16x64) per head.
  - den8 = 8*(n + qKsq/(2*DH)) comes out of a single scan: the per-query
    "+8" and the 1/16 = 8/(2*DH) scale live in the reduction matmuls'
    selector weights, so there is no separate n-add pass.
  - y = mv + qkv * (1/den8): reciprocal on DVE, row broadcast on GPSIMD.

v3 on top of v2:
  - Packed projections: q/k/v share 3 psum tiles (q slots ride in the
    unused 32-aligned lanes of the v/k tiles), 18 matmuls/chunk vs 24.
    Single strided copies reassemble canonical qt/ck layouts.
  - One merged hT DMA per chunk + one packed weight "wall" param: each
    DMACopy costs a fixed ~625ns on the shared HWDGE device, so fewer,
    bigger transfers (chunk 0 stays split + slab-major so PE starts at
    ~3.5us instead of ~5.8us).
  - Out DMAs paired two blocks per transfer; the last two blocks go per
    384-half so the tail transfer starts as early as possible.
  - Last chunk's scans/centering run in two 256-col halves so the final
    emit_blocks starts on half-0 (Tile deps are slice-granular).
  - cgsb[:, NB-1] is never read: its C/G matmuls and snapshot are skipped.

Layout: every sliced per-head partition base is 32-aligned (head h rows sit
at [32h, 32h+16)); walrus rejects non-32-aligned partition bases on
compute-engine APs.  The denominator pipeline therefore also keeps its
per-head rows at 32h (s3/den8/r3 span partitions 0..65, rows {0,32,64}
live).  Matmul operands keep lhsT/rhs partition bases equal (PE
tile_position rule); offset transposes use identity slices idt[32h:, 32h:].
"""

import numpy as np

import concourse.bacc as bacc
import concourse.mybir as mybir
import concourse.tile as tile
from concourse.masks import make_identity

F32 = mybir.dt.float32
F32R = mybir.dt.float32r
BF16 = mybir.dt.bfloat16
ADD = mybir.AluOpType.add
BYPASS = mybir.AluOpType.bypass

B, L, D = 2, 2048, 768
H, F, DH = 12, 16, 64
HPC = 3                 # heads per core
NCORES = 8
NB = L // 128           # 16 key blocks
NQ = L // 512           # 4 query chunks
QC = 512                # query chunk size
KB = 128                # key block size
PH = 96                 # padded per-head partition span (3 heads x 32)
SH = 65                 # den/s partition span (rows 32h live, h<3)
CW = DH + F             # per-head [G | C] width = 80
S16 = 8.0 / (2.0 * DH)  # 1/16: folds the 8x den scale and 1/(2*DH)


def build_nc(dbg=False):
    nc = bacc.Bacc("TRN2", target_bir_lowering=False, debug=False)

    hT = nc.declare_dram_parameter("hT", [D, L], BF16, isOutput=False)
    # packed projection wall (3 psum tiles of 128 cols each):
    #   cols   0:128  tile1 = v heads 0,1
    #   cols 128:256  tile2 = v head 2 at +0:64, q_h0 at +64:80, q_h1 at
    #                 +96:112
    #   cols 256:384  tile3 = k_h at +32h:+32h+16, q_h2 at +96:112
    #   (an AP may not cross the 64-partition boundary unless it starts at
    #   0 or 64, so k sits at base 0 like the canonical ck layout)
    wall = nc.declare_dram_parameter("wall", [D, 3 * 128], BF16,
                                     isOutput=False)
    wo = nc.declare_dram_parameter("wo", [HPC * DH, D], BF16, isOutput=False)
    invn = nc.declare_dram_parameter("invn", [128, L], BF16, isOutput=False)
    out_e = nc.declare_dram_parameter("out", [L, D], BF16, isOutput=True)
    if dbg:
        d_qt = nc.declare_dram_parameter("d_qt", [PH, L], BF16, isOutput=True)
        d_ck = nc.declare_dram_parameter("d_ck", [PH, L], BF16, isOutput=True)
        d_cv = nc.declare_dram_parameter("d_cv", [128, L], BF16,
                                         isOutput=True)
        d_mv = nc.declare_dram_parameter("d_mv", [128, L], BF16,
                                         isOutput=True)
        d_den = nc.declare_dram_parameter("d_den", [SH, L], F32,
                                          isOutput=True)
        d_y = nc.declare_dram_parameter("d_y", [128, L], BF16, isOutput=True)
        d_qkv = nc.declare_dram_parameter("d_qkv", [128, L], BF16,
                                          isOutput=True)
        d_cg = nc.declare_dram_parameter("d_cg", [PH, NB, CW], BF16,
                                         isOutput=True)

    with tile.TileContext(nc) as tc:
        with (
            tc.tile_pool(name="const", bufs=1) as cpool,
            tc.tile_pool(name="big", bufs=1) as bpool,
        ):
            # ---------- persistent big tiles ----------
            qt = bpool.tile([PH, L], BF16, tag="qt")
            ck = bpool.tile([PH, L], BF16, tag="ck")    # kT, centered in place
            ksc = bpool.tile([PH, L], BF16, tag="ksc")  # raw k cumsum
            cvh = bpool.tile([128, L], BF16, tag="cvh")  # vT h0,h1 -> cv
            cvl = bpool.tile([64, L], BF16, tag="cvl")   # vT h2 -> cv
            vsh = bpool.tile([128, L], BF16, tag="vsh")  # raw v cumsum
            vsl = bpool.tile([64, L], BF16, tag="vsl")
            mvh = bpool.tile([128, L], BF16, tag="mvh")  # mean_vT
            mvl = bpool.tile([64, L], BF16, tag="mvl")
            ckcv = bpool.tile([128, NB, HPC * CW], BF16, tag="ckcv")
            cgsb = bpool.tile([PH, NB, CW], BF16, tag="cgsb")
            den8 = bpool.tile([SH, L], F32, tag="den8")
            r3 = bpool.tile([SH, L], F32, tag="r3")
            yth = bpool.tile([128, L], BF16, tag="yth")
            ytl = bpool.tile([64, L], BF16, tag="ytl")
            squ = bpool.tile([128, QC], BF16, tag="squ")  # row 96 == 8.0

            # ---------- weights (SP-issued; gate slabs first) ----------
            wall_sb = cpool.tile([128, 6, 3 * 128], BF16)
            wall_r = wall[:].rearrange("(c p) f -> p c f", p=128)
            nc.sync.dma_start(wall_sb[:, 0:2, :], wall_r[:, 0:2, :])

            # ---------- constants (gpsimd; overlaps DMA) ----------
            idt = cpool.tile([128, 128], BF16)
            make_identity(nc, idt[:])
            mask4 = cpool.tile([128, 4, KB], BF16)
            nc.gpsimd.memset(mask4[:], 1.0)
            nc.gpsimd.affine_select(
                out=mask4[:], in_=mask4[:],
                compare_op=mybir.AluOpType.is_ge, fill=0.0,
                base=0, pattern=[[0, 4], [1, KB]], channel_multiplier=-1,
            )
            sels = []
            for h in range(HPC):
                sel = cpool.tile([128, SH], BF16, name=f"sel{h}",
                                 tag=f"sel{h}")
                nc.gpsimd.memset(sel[:], 0.0)
                nc.gpsimd.memset(sel[:, 32 * h:32 * h + 1], S16)
                sels.append(sel)
            sel97 = cpool.tile([128, SH], BF16)
            nc.gpsimd.memset(sel97[:], 0.0)
            for h in range(HPC):
                nc.gpsimd.memset(
                    sel97[32 * h:32 * h + F, 32 * h:32 * h + 1], S16)
                nc.gpsimd.memset(sel97[96:97, 32 * h:32 * h + 1], 1.0)
            nc.gpsimd.memset(squ[96:97, :], 8.0)

            # ---------- phase 1: proj + center + transpose + C/G ----------
            with (
                tc.tile_pool(name="ht", bufs=3) as htpool,
                tc.tile_pool(name="mk", bufs=3) as mkpool,
                tc.tile_pool(name="pp", bufs=4, space="PSUM") as pp,
                tc.tile_pool(name="ptr", bufs=2, space="PSUM") as ptr,
                tc.tile_pool(name="pcg", bufs=1, space="PSUM") as pcg,
            ):
                cgps = pcg.tile([PH, CW], F32)
                nc.vector.memset(cgps[:], 0.0)
                invn_sb = cpool.tile([128, L], BF16)

                def emit_blocks(jq):
                    # transposes into [keys, cv|ck] + C/G updates; deferred
                    # one chunk so PE never waits on this chunk's centering
                    for b in range(4):
                        gb = 4 * jq + b
                        cs = slice(KB * gb, KB * (gb + 1))
                        pt = ptr.tile([128, HPC * CW], BF16, name="pt",
                                      tag="pt")
                        for h in range(HPC):
                            if h < 2:
                                src, hb = cvh[64 * h:64 * (h + 1), cs], 64 * h
                            else:
                                src, hb = cvl[:, cs], 0
                            nc.tensor.transpose(
                                pt[:, CW * h:CW * h + DH], src,
                                idt[hb:hb + 64, hb:hb + 64])
                            nc.tensor.transpose(
                                pt[:, CW * h + DH:CW * (h + 1)],
                                ck[32 * h:32 * h + F, cs],
                                idt[32 * h:32 * h + F, 32 * h:32 * h + F])
                        if gb % 2 == 0:
                            nc.scalar.copy(ckcv[:, gb, :], pt[:])
                        else:
                            nc.vector.tensor_copy(ckcv[:, gb, :], pt[:])
                        if gb == NB - 1:
                            continue  # cgsb[:, NB-1] is never read
                        for h in range(HPC):
                            hs = slice(32 * h, 32 * h + F)
                            nc.tensor.matmul(
                                cgps[hs, :],
                                ckcv[:, gb, CW * h + DH:CW * (h + 1)],
                                ckcv[:, gb, CW * h:CW * (h + 1)],
                                start=(gb == 0 and h == 0),
                                stop=(gb == NB - 2 and h == 2),
                                skip_group_check=True)
                        nc.scalar.copy(cgsb[:, gb, :], cgps[:])

                for jq in range(NQ):
                    qs = slice(QC * jq, QC * (jq + 1))
                    ht_t = htpool.tile([128, 6, QC], BF16, name="ht",
                                       tag="ht")
                    ht_r = hT[:, qs].rearrange("(c p) f -> p c f", p=128)
                    if jq == 0:
                        # split the first chunk so matmuls can start on the
                        # first two slabs while the rest transfers
                        nc.sync.dma_start(ht_t[:, 0:2, :], ht_r[:, 0:2, :])
                        nc.sync.dma_start(ht_t[:, 2:6, :], ht_r[:, 2:6, :])
                        nc.sync.dma_start(wall_sb[:, 2:6, :],
                                          wall_r[:, 2:6, :])
                        nc.sync.dma_start(invn_sb[:], invn[:])
                    else:
                        nc.sync.dma_start(ht_t[:], ht_r)
                    hts = [ht_t[:, k, :] for k in range(6)]
                    # projections: tile3 (k + q_h2) first — its ck copy
                    # gates the scan chain.  chunk 0 goes slab-major so PE
                    # has 6 runnable matmuls as soon as slabs 0:2 land.
                    p3 = pp.tile([128, QC], F32, name="p3", tag="pa")
                    p1 = pp.tile([128, QC], F32, name="p1", tag="pa")
                    p2 = pp.tile([128, QC], F32, name="p2", tag="pa")
                    mms = [(p3, 256, 384), (p1, 0, 128), (p2, 128, 256)]
                    if jq == 0:
                        order = [(t, k) for k in range(6) for t in range(3)]
                    else:
                        order = [(t, k) for t in range(3) for k in range(6)]
                    for t, k in order:
                        pt_, c0, c1 = mms[t]
                        nc.tensor.matmul(pt_[:], wall_sb[:, k, c0:c1],
                                         hts[k], start=(k == 0),
                                         stop=(k == 5))
                        if k == 5 and t == 0:
                            nc.scalar.copy(ck[:, qs], p3[0:96, :])
                        elif k == 5 and t == 1:
                            nc.scalar.copy(cvh[:, qs], p1[:])
                        elif k == 5 and t == 2:
                            nc.vector.tensor_copy(cvl[:, qs], p2[0:64, :])
                            # widened to pull in zero rows so
                            # qt[48:64)/[80:96) are defined (squ-mul reads
                            # all 96 qt rows)
                            nc.scalar.copy(qt[0:64, qs], p2[64:128, :])
                            nc.scalar.copy(qt[64:96, qs], p3[96:128, :])

                    # chained scans + centering.  last chunk: two 256-col
                    # halves so emit_blocks(NQ-1)'s first blocks (which
                    # read only the first half) start ~1us earlier
                    nparts = 2 if jq == NQ - 1 else 1
                    pw = QC // nparts
                    mk = mkpool.tile([PH, QC], BF16, name="mk", tag="mk")
                    for pi in range(nparts):
                        c0 = QC * jq + pw * pi
                        ps_ = slice(c0, c0 + pw)
                        ms_ = slice(pw * pi, pw * (pi + 1))
                        ik = (0.0 if jq == 0 and pi == 0
                              else ksc[:, c0 - 1:c0])
                        nc.vector.tensor_tensor_scan(
                            ksc[:, ps_], ck[:, ps_], ck[:, ps_], ik,
                            ADD, BYPASS)
                        ih = (0.0 if jq == 0 and pi == 0
                              else vsh[:, c0 - 1:c0])
                        nc.vector.tensor_tensor_scan(
                            vsh[:, ps_], cvh[:, ps_], cvh[:, ps_], ih,
                            ADD, BYPASS)
                        il = (0.0 if jq == 0 and pi == 0
                              else vsl[:, c0 - 1:c0])
                        nc.vector.tensor_tensor_scan(
                            vsl[:, ps_], cvl[:, ps_], cvl[:, ps_], il,
                            ADD, BYPASS)
                        nc.vector.tensor_mul(mk[:, ms_], ksc[:, ps_],
                                             invn_sb[0:PH, ps_])
                        nc.vector.tensor_sub(ck[:, ps_], ck[:, ps_],
                                             mk[:, ms_])
                        nc.vector.tensor_mul(mvh[:, ps_], vsh[:, ps_],
                                             invn_sb[:, ps_])
                        nc.vector.tensor_sub(cvh[:, ps_], cvh[:, ps_],
                                             mvh[:, ps_])
                        nc.gpsimd.tensor_mul(mvl[:, ps_], vsl[:, ps_],
                                             invn_sb[0:64, ps_])
                        nc.gpsimd.tensor_sub(cvl[:, ps_], cvl[:, ps_],
                                             mvl[:, ps_])
                    if jq > 0:
                        emit_blocks(jq - 1)
                emit_blocks(NQ - 1)
                wo_h = cpool.tile([128, D], BF16)
                nc.sync.dma_start(wo_h[:], wo[0:128, :])
                wo_l = cpool.tile([64, D], BF16)
                nc.sync.dma_start(wo_l[:], wo[128:192, :])

            # ---------- phase 2: scores + denominators + output ----------
            with (
                tc.tile_pool(name="mqk", bufs=3) as mqkpool,
                tc.tile_pool(name="sqp", bufs=3) as sqpool,
                tc.tile_pool(name="qkv16", bufs=4) as qkv16pool,
                tc.tile_pool(name="rt", bufs=3) as rtpool,
                tc.tile_pool(name="rb", bufs=3) as rbpool,
                tc.tile_pool(name="osb", bufs=3) as opool,
                tc.tile_pool(name="osb1", bufs=2) as opool1,
                tc.tile_pool(name="pqk", bufs=2, space="PSUM") as pqk,
                tc.tile_pool(name="pqkv", bufs=2, space="PSUM") as pqkv,
                tc.tile_pool(name="pu", bufs=1, space="PSUM") as pu,
                tc.tile_pool(name="ps3", bufs=1, space="PSUM") as ps3,
                tc.tile_pool(name="po", bufs=2, space="PSUM") as po,
            ):
                u96 = pu.tile([PH, QC], F32)
                nc.vector.memset(u96[:], 0.0)

                def u_mms(jq):
                    # u = C q for chunk jq, emitted one chunk ahead
                    first_u = True
                    for h in range(HPC):
                        hs = slice(32 * h, 32 * h + F)
                        for b in range(4):
                            gb = 4 * jq + b
                            if gb > 0:
                                qcs = slice(KB * gb, KB * (gb + 1))
                                nc.tensor.matmul(
                                    u96[hs, KB * b:KB * (b + 1)],
                                    cgsb[hs, gb - 1, DH:CW], qt[hs, qcs],
                                    start=first_u,
                                    stop=(h == 2 and b == 3),
                                    skip_group_check=True)
                                first_u = False

                pending = []  # deferred out-proj pieces of the prior chunk

                osbs = {}

                def emit_outproj(lb, half, eng):
                    # blocks pair into one [256, D] DMA; last two blocks
                    # stay single so the tail DMA starts sooner
                    ls = slice(KB * lb, KB * (lb + 1))
                    n0 = 384 * half
                    single = lb >= NB - 2
                    op = po.tile([128, 384], F32, name="op", tag="op")
                    nc.tensor.matmul(op[:], yth[:, ls], wo_h[:, n0:n0 + 384],
                                     start=True, stop=False)
                    nc.tensor.matmul(op[:], ytl[:, ls], wo_l[:, n0:n0 + 384],
                                     start=False, stop=True)
                    if single:
                        if lb not in osbs:
                            osbs[lb] = opool1.tile([128, D], BF16,
                                                   name="osb1", tag="osb1")
                        osb = osbs[lb]
                        dst = osb[:, n0:n0 + 384]
                    else:
                        if lb // 2 not in osbs:
                            osbs[lb // 2] = opool.tile([128, 2, D], BF16,
                                                       name="osb", tag="osb")
                        osb = osbs[lb // 2]
                        dst = osb[:, lb % 2, n0:n0 + 384]
                    if eng == 0:
                        nc.scalar.copy(dst, op[:])
                    else:
                        nc.vector.tensor_copy(dst, op[:])
                    if single:
                        # tail blocks: DMA each 384-half as soon as it is
                        # staged so the last transfer starts earlier
                        nc.sync.dma_start(out_e[ls, n0:n0 + 384],
                                          osb[:, n0:n0 + 384])
                    elif half == 1 and lb % 2 == 1:
                        nc.sync.dma_start(
                            out_e[KB * (lb - 1):KB * (lb + 1),
                                  :].rearrange("(c p) f -> p c f",
                                               p=128),
                            osb[:])

                def drain(npop):
                    for _ in range(min(npop, len(pending))):
                        pending.pop(0)()

                for jq in range(NQ):
                    qs = slice(QC * jq, QC * (jq + 1))
                    s3 = ps3.tile([SH, QC], F32, name="s3", tag="s3")
                    qk16s = [qkv16pool.tile([64, QC], BF16, name=f"qk16{h}",
                                            tag=f"q{h}") for h in range(HPC)]
                    qkps, qkvps, mqs = {}, {}, {}

                    def scores(h):
                        # one start=True per psum bank: PSUM zeroing is
                        # 2KB-bank granular, a second start wipes siblings
                        hs = slice(32 * h, 32 * h + F)
                        qkp = pqk.tile([128, 4, KB], F32, name="qkp",
                                       tag="qk")
                        qkvp = pqkv.tile([64, QC], F32, name="qkvp",
                                         tag="qkv")
                        qkps[h], qkvps[h] = qkp, qkvp
                        first_hist = True
                        for b in range(4):
                            gb = 4 * jq + b
                            qcs = slice(KB * gb, KB * (gb + 1))
                            nc.tensor.matmul(qkp[:, b, :], ck[hs, qcs],
                                             qt[hs, qcs], start=(b == 0),
                                             stop=(b == 3),
                                             skip_group_check=True)
                            if gb > 0:
                                nc.tensor.matmul(
                                    qkvp[:, KB * b:KB * (b + 1)],
                                    cgsb[hs, gb - 1, 0:DH], qt[hs, qcs],
                                    start=first_hist, stop=False,
                                    skip_group_check=True)
                                first_hist = False

                    def mask_sq(h):
                        mq = mqkpool.tile([128, 4, KB], BF16, name="mq",
                                          tag="mqk")
                        mqs[h] = mq
                        nc.vector.tensor_mul(mq[:], qkps[h][:], mask4[:])
                        sqt = sqpool.tile([128, 4, KB], BF16, name="sqt",
                                          tag="sq")
                        if h < 2:
                            # Pool has slack here and h0/h1's squares are
                            # off the den critical chain (den needs all
                            # heads + u); h2's stays on DVE (chain-last)
                            nc.gpsimd.tensor_mul(sqt[:], mq[:], mq[:])
                        else:
                            nc.vector.tensor_mul(sqt[:], mq[:], mq[:])
                        return sqt

                    def reduce_h(h, sqt):
                        qkvp, mq = qkvps[h], mqs[h]
                        for b in range(4):
                            gb = 4 * jq + b
                            nc.tensor.matmul(
                                s3[:, KB * b:KB * (b + 1)], sels[h][:],
                                sqt[:, b, :], start=(h == 0 and b == 0),
                                stop=False, skip_group_check=True)
                            nc.tensor.matmul(
                                qkvp[:, KB * b:KB * (b + 1)],
                                ckcv[:, gb, CW * h:CW * h + DH],
                                mq[:, b, :], start=False,
                                stop=(b == 3), skip_group_check=True)
                        nc.scalar.copy(qk16s[h][:], qkvp[:])

                    last = jq == NQ - 1
                    scores(0)
                    drain(0 if last else 2)
                    scores(1)
                    sq0 = mask_sq(0)
                    sq1 = mask_sq(1)
                    reduce_h(0, sq0)
                    scores(2)
                    drain(0 if last else 2)
                    sq2 = mask_sq(2)
                    reduce_h(1, sq1)
                    reduce_h(2, sq2)
                    u_mms(jq)
                    nc.vector.tensor_mul(squ[0:PH, :], u96[:], qt[:, qs])
                    drain(0 if last else 4)
                    for b in range(4):
                        nc.tensor.matmul(
                            s3[:, KB * b:KB * (b + 1)], sel97[0:97, :],
                            squ[0:97, KB * b:KB * (b + 1)], start=False,
                            stop=(b == 3), skip_group_check=True)
                    if last:
                        # keep the prior chunk's out-proj pieces for the
                        # final tail, where PE otherwise idles
                        drain(8)
                    # den8 = cumsum(s + 8) = 8n + 8*qKsq/(2DH); r = 1/den8
                    # half-chunk passes: shorter per-link chain, and the
                    # final out-proj pieces start as soon as ready
                    nhalf = 2
                    hw_ = QC // nhalf
                    for hf in range(nhalf):
                        c0 = QC * jq + hw_ * hf
                        hqs = slice(c0, c0 + hw_)
                        sqs = slice(hw_ * hf, hw_ * (hf + 1))
                        init = (0.0 if jq == 0 and hf == 0
                                else den8[:, c0 - 1:c0])
                        nc.vector.tensor_tensor_scan(
                            den8[:, hqs], s3[:, sqs], invn_sb[0:SH, hqs],
                            init, ADD, BYPASS)
                        nc.vector.reciprocal_approx_fast(out=r3[:, hqs],
                                                         in_=den8[:, hqs])
                        for h in range(HPC):
                            rt = rtpool.tile([1, QC], BF16, name="rt",
                                             tag="rt")
                            if h == 1:
                                nc.vector.tensor_copy(
                                    rt[:, sqs], r3[32 * h:32 * h + 1, hqs])
                            else:
                                nc.scalar.copy(rt[:, sqs],
                                               r3[32 * h:32 * h + 1, hqs])
                            # partition_broadcast: base-0 in and out APs;
                            # DVE tensor-tensor: equal input base partitions
                            rb = rbpool.tile([64, QC], BF16, name="rb",
                                             tag="rb")
                            nc.gpsimd.partition_broadcast(rb[:, sqs],
                                                          rt[0:1, sqs])
                            if h < 2:
                                nc.vector.tensor_mul(
                                    yth[64 * h:64 * (h + 1), hqs],
                                    qk16s[h][:, sqs], rb[:, sqs])
                            else:
                                nc.vector.tensor_mul(ytl[:, hqs],
                                                     qk16s[h][:, sqs],
                                                     rb[:, sqs])
                                nc.vector.tensor_add(ytl[:, hqs],
                                                     ytl[:, hqs],
                                                     mvl[:, hqs])
                        nc.vector.tensor_add(yth[:, hqs], yth[:, hqs],
                                             mvh[:, hqs])
                        if last:
                            for b in range(2):
                                lb = 4 * jq + 2 * hf + b
                                emit_outproj(lb, 0, 0)
                                emit_outproj(lb, 1, 2)
                    if dbg:
                        nc.sync.dma_start(d_qkv[0:64, qs], qk16s[0][:])
                        nc.sync.dma_start(d_qkv[64:128, qs], qk16s[1][:])
                    if not last:
                        engs = [0, 0, 0, 0, 0, 0, 0, 2]
                        for i, (lb, half) in enumerate(
                                (4 * jq + b, half)
                                for b in range(4) for half in range(2)):
                            pending.append(
                                (lambda lb=lb, half=half, e=engs[i]:
                                 emit_outproj(lb, half, e)))
                drain(len(pending))
                if dbg:
                    nc.sync.dma_start(d_qt[:], qt[:])
                    nc.sync.dma_start(d_ck[:], ck[:])
                    nc.sync.dma_start(d_cv[:], cvh[:])
                    nc.sync.dma_start(d_mv[:], mvh[:])
                    nc.sync.dma_start(d_den[:], den8[:])
                    nc.sync.dma_start(d_y[:], yth[:])
                    nc.sync.dma_start(d_cg[:], cgsb[:])

    nc.compile()
    return nc


_CACHED = {}


def _shard_inputs(hidden_states, Wq, Wk, Wv, Wo):
    import ml_dtypes
    bf16 = ml_dtypes.bfloat16

    n = np.arange(1, L + 1, dtype=np.float32)
    invn = np.ascontiguousarray(
        np.broadcast_to(1.0 / n, (128, L))).astype(bf16)

    in_maps = []
    for c in range(NCORES):
        b, hg = c // 4, c % 4
        hs = slice(HPC * F * hg, HPC * F * (hg + 1))
        vs = slice(HPC * DH * hg, HPC * DH * (hg + 1))
        wq_c = np.asarray(Wq[:, hs], dtype=np.float32)
        wk_c = np.asarray(Wk[:, hs], dtype=np.float32)
        wv_c = np.asarray(Wv[:, vs], dtype=np.float32)
        wall = np.zeros((D, 3 * 128), dtype=np.float32)
        wall[:, 0:128] = wv_c[:, 0:128]              # tile1: v h0,h1
        wall[:, 128:192] = wv_c[:, 128:192]          # tile2: v h2
        wall[:, 192:208] = wq_c[:, 0:16]             # tile2: q h0 @ +64
        wall[:, 224:240] = wq_c[:, 16:32]            # tile2: q h1 @ +96
        wall[:, 352:368] = wq_c[:, 32:48]            # tile3: q h2 @ +96
        for h in range(HPC):
            wall[:, 256 + 32 * h:256 + 32 * h + 16] = \
                wk_c[:, F * h:F * (h + 1)]           # tile3: k_h @ +32h
        in_maps.append({
            "hT": np.ascontiguousarray(
                np.asarray(hidden_states[b], dtype=np.float32).T
            ).astype(bf16),
            "wall": np.ascontiguousarray(wall).astype(bf16),
            "wo": np.ascontiguousarray(
                np.asarray(Wo[vs, :], dtype=np.float32)).astype(bf16),
            "invn": invn,
        })
    return in_maps


def kernel(hidden_states, Wq, Wk, Wv, Wo, _trace=False):
    from concourse.bass_utils import run_bass_kernel_spmd
    if "nc" not in _CACHED:
        _CACHED["nc"] = build_nc()
    in_maps = _shard_inputs(np.asarray(hidden_states), np.asarray(Wq),
                            np.asarray(Wk), np.asarray(Wv), np.asarray(Wo))
    res = run_bass_kernel_spmd(_CACHED["nc"], in_maps,
                               core_ids=list(range(NCORES)), trace=_trace)
    out = np.zeros((B, L, D), dtype=np.float32)
    for c in range(NCORES):
        out[c // 4] += np.asarray(res.results[c]["out"]).astype(np.float32)
    if _trace:
        kernel._last_exec_time_ns = res.exec_time_ns
        kernel._last_profile = res
    return out

